# revision 17
# baseline (speedup 1.0000x reference)
"""Multi-head causal attention (B=8, S=1024, C=1024, H=16, dk=dv=64) on 8 trn2 cores.

Sharding: data-parallel over batch. Each NeuronCore processes one batch element
end-to-end (projections + attention + output projection); no collectives.

Schedule (single instruction stream, engines run async, in-order per queue):
  K proj -> V proj -> Q proj (q-chunk 0) ->
  attention q-chunk 0  (interleaved with Q proj q-chunk 1 on the PE) ->
  attention q-chunk 1  (interleaved with output proj of q-chunk 0) ->
  output proj q-chunk 1.
The attention inner loop is ACT(exp)-limited, so dense projection matmuls are
threaded between heads to keep the PE streaming (and at max p-state).

Per-core layout:
  xq/xk/xv = X^T [C, S] bf16 (host-transposed), packed weights
  wq/wk [C, H*DK] (wq pre-scaled by 1/sqrt(dk)), wv [C, H*DV], wo [H*DV, C].

  QT = wq.T @ xq -> [H*DK, S]  (head h on partitions (h%2)*64..+64 of m-tile h//2)
  KT = wk.T @ xk -> [H*DK, S]  (same packing; St matmuls contract K=64 at a
                                partition offset - no zero-padding needed)
  V  = xv.T @ wv -> [S, H*DV]  (+ ones column per head for the softmax denom)
  per head, q-chunk: St[t,q] tiles are packed into 2-bank PSUM tiles so one
  ACT exp instruction covers up to 3 t-tiles (amortizes the ~185ns ACT bubble);
  diagonal blocks masked post-exp by a bf16 triangular multiply on DVE.
  O^T/r: denominator row -> DVE reciprocal -> GpSimd partition-broadcast ->
  GpSimd multiply into oT (keeps the DVE queue free for masks).
  Y = concat(O)^T-contract @ wo -> [S, C] f32
"""

import math
import os
import sys

import numpy as np

try:
    import concourse.bass as bass
except ImportError:  # make concourse importable in a bare grading dir
    for _p in ("/opt/trn_rl_repo", os.path.expanduser("~/.axon_site/_ro/trn_rl_repo")):
        if os.path.isdir(_p) and _p not in sys.path:
            sys.path.insert(0, _p)
    import concourse.bass as bass

from contextlib import ExitStack

import ml_dtypes

import concourse.mybir as mybir
import concourse.tile as tile
from concourse import bacc
from concourse.bass_utils import run_bass_kernel_spmd


def _setup_act_tables():
    """Pin the ACT function table to the set that covers exp+copy+identity
    so the kernel never reloads LUTs mid-flight."""
    import json
    import shutil
    import tempfile

    import concourse.hw_specs as hw_specs
    from concourse import bacc as _bacc

    if os.environ.get("BASS_ACT_ROOT_JSON_PATH"):
        return  # already configured
    from neuronxcc.driver.Job import Job

    orig = os.path.join(
        Job.getPackageDir(), "pwp", "pwp_bin_trainium", "act_info.json"
    )
    assert os.path.isfile(orig), orig
    dst = os.path.join(tempfile.gettempdir(), "mha_act_tables")
    if not os.path.isdir(dst):
        tmp = dst + ".tmp"
        shutil.rmtree(tmp, ignore_errors=True)
        shutil.copytree(os.path.dirname(orig), tmp)
        with open(os.path.join(tmp, "act_info.json")) as f:
            info = json.load(f)
        sets = info["act_func_sets"]
        want = [s for s in sets if s["name"] == "natural_log_exp_and_others"]
        rest = [s for s in sets if s["name"] != "natural_log_exp_and_others"]
        info["act_func_sets"] = want + rest
        with open(os.path.join(tmp, "act_info.json"), "w") as f:
            json.dump(info, f)
        os.replace(tmp, dst)
    path = os.path.join(dst, "act_info.json")
    os.environ["BASS_ACT_ROOT_JSON_PATH"] = path

    def patched(module_arch):
        with open(path) as af:
            act_info = json.load(af)
        return {
            ent["name"]: {
                mybir.ActivationFunctionType.from_pwp(v) for v in ent["act"].keys()
            }
            for ent in act_info["act_func_sets"]
        }

    hw_specs.get_activation_tables = patched
    _bacc.get_activation_tables = patched
    from concourse import bass_interp as _bi

    _bi.get_activation_tables = patched


B, S, C = 8, 1024, 1024
H, DK, DV = 16, 64, 64
P = 128
NT = 8  # number of 128-tiles along S / C / H*DK
CH = 512  # q-chunk (one PSUM bank of fp32)
NCH = S // CH

FP = mybir.dt.float32
BF = mybir.dt.bfloat16
BF_NP = ml_dtypes.bfloat16
AFT = mybir.ActivationFunctionType
ALU = mybir.AluOpType


def _st_groups(jc):
    """Pack the causal-trimmed St tiles of q-chunk jc into PSUM groups.

    Returns a list of groups; each group is a list of (i, seg_off, width)
    fitting in one [P, 1024] 2-bank PSUM tile (exp'd by a single ACT op).
    """
    n_i = min(NT, (jc + 1) * CH // P)
    segs = []
    for i in range(n_i):
        off = max(0, i * P - jc * CH)
        segs.append((i, CH - off))
    if jc == 0:
        # widths 512,384,256,128 -> (i0,i1,i3) = 1024 exact, (i2) alone
        return [[(0, 0, 512), (1, 512, 384), (3, 896, 128)], [(2, 0, 256)]]
    # jc == 1: widths 512x5,384,256,128
    return [
        [(0, 0, 512), (1, 512, 512)],
        [(2, 0, 512), (3, 512, 512)],
        [(4, 0, 512), (5, 512, 384), (7, 896, 128)],
        [(6, 0, 256)],
    ]


def build_nc(zero_bias: bool) -> bass.Bass:
    _setup_act_tables()
    nc = bacc.Bacc()

    xq = nc.dram_tensor("xq", [C, S], BF, kind="ExternalInput")
    xk = nc.dram_tensor("xk", [C, S], BF, kind="ExternalInput")
    xv = nc.dram_tensor("xv", [C, S], BF, kind="ExternalInput")
    wq = nc.dram_tensor("wq", [C, H * DK], BF, kind="ExternalInput")
    wk = nc.dram_tensor("wk", [C, H * DK], BF, kind="ExternalInput")
    wv = nc.dram_tensor("wv", [C, H * DV], BF, kind="ExternalInput")
    wo = nc.dram_tensor("wo", [H * DV, C], BF, kind="ExternalInput")
    if not zero_bias:
        bqd = nc.dram_tensor("bq", [P, NT], FP, kind="ExternalInput")
        bkd = nc.dram_tensor("bk", [P, NT], FP, kind="ExternalInput")
        bvd = nc.dram_tensor("bv", [P, H * DV], FP, kind="ExternalInput")
        bod = nc.dram_tensor("bo", [1, C], FP, kind="ExternalInput")
    y = nc.dram_tensor("y", [S, C], FP, kind="ExternalOutput")

    # binary causal mask [t,q] (1 iff t<=q), bf16, multiplied post-exp
    tri_d = nc.inline_tensor(
        np.triu(np.ones((P, P), np.float32)).astype(BF_NP), "tri"
    )

    xq_r = xq.rearrange("(ko p) s -> p ko s", p=P)
    xk_r = xk.rearrange("(ko p) s -> p ko s", p=P)
    xv_r = xv.rearrange("(ko p) s -> p ko s", p=P)
    wq_r = wq.rearrange("(ko p) m -> p ko m", p=P)
    wk_r = wk.rearrange("(ko p) m -> p ko m", p=P)
    wv_r = wv.rearrange("(ko p) m -> p ko m", p=P)
    wo_r = wo.rearrange("(ko p) c -> p ko c", p=P)
    y_r = y.rearrange("(mo p) c -> p mo c", p=P)

    with tile.TileContext(nc) as tc, ExitStack() as octx:
        const = octx.enter_context(tc.tile_pool(name="const", bufs=1))
        big = octx.enter_context(tc.tile_pool(name="big", bufs=1))
        wxp = octx.enter_context(tc.tile_pool(name="wx", bufs=2))
        pchp = octx.enter_context(tc.tile_pool(name="pch", bufs=6))
        smalls = octx.enter_context(tc.tile_pool(name="smalls", bufs=3))
        ypool = octx.enter_context(tc.tile_pool(name="y", bufs=3))
        # PSUM: pairs 2x2 banks + pv 2 + out 2 = 8 banks exactly
        ps_pair = octx.enter_context(tc.tile_pool(name="ps_pair", bufs=2, space="PSUM"))
        ps_pv = octx.enter_context(tc.tile_pool(name="ps_pv", bufs=2, space="PSUM"))
        ps_out = octx.enter_context(tc.tile_pool(name="ps_out", bufs=2, space="PSUM"))

        tri_sb = const.tile([P, P], BF, tag="tri")
        nc.sync.dma_start(tri_sb, tri_d[:])
        if not zero_bias:
            bq_sb = const.tile([P, NT], FP, tag="bq")
            nc.sync.dma_start(bq_sb, bqd[:])
            bk_sb = const.tile([P, NT], FP, tag="bk")
            nc.sync.dma_start(bk_sb, bkd[:])
            bv_sb = const.tile([P, H * DV], FP, tag="bv")
            nc.sync.dma_start(bv_sb, bvd[:])
            bo_sb = const.tile([1, C], FP, tag="bo")
            nc.sync.dma_start(bo_sb, bod[:])
            borep_sb = const.tile([P, C], FP, tag="borep")
            nc.gpsimd.partition_broadcast(borep_sb, bo_sb)

        qT_sb = big.tile([P, NT, S], BF, tag="qT")
        kT_sb = big.tile([P, NT, S], BF, tag="kT")
        # per-head PV lhsT is [t, 128]: col 0 = ones (denominator -> PSUM
        # partition 0, the only base custom-DVE/broadcast ops accept),
        # cols 64:128 = V (O^T rows -> aligned partition window 64:128)
        v_sb = big.tile([P, NT, H, P], BF, tag="v")
        oT_sb = big.tile([P, NT, S], BF, tag="oT")
        wo_sb = big.tile([P, NT, C], BF, tag="wo")

        nc.vector.memset(v_sb[:, :, :, 0], 1.0)
        nc.vector.memset(v_sb[:, :, :, 1:DV], 0.0)

        evac_flip = [0]

        def evac_copy(dst, src, act_ok=True):
            # alternate DVE / ACT while ACT is idle (projection phase);
            # DVE-only inside attention windows where ACT is the bottleneck
            evac_flip[0] ^= 1
            if evac_flip[0] and act_ok:
                nc.scalar.copy(dst, src)
            else:
                nc.vector.tensor_copy(out=dst, in_=src)

        def proj_chain(w_sb, x_sb, m, n, pool, name):
            ps = pool.tile([P, CH], FP, tag="c512", name=name)
            for kc in range(NT):
                nc.tensor.matmul(
                    ps,
                    w_sb[:, kc, m * P : (m + 1) * P],
                    x_sb[:, kc, n * CH : (n + 1) * CH],
                    start=(kc == 0),
                    stop=(kc == NT - 1),
                )
            return ps

        # ---------------- K projection ----------------
        wk_sb = wxp.tile([P, NT, H * DK], BF, tag="w", name="wk_sb")
        xk_sb = wxp.tile([P, NT, S], BF, tag="x", name="xk_sb")
        for kc in range(NT):
            nc.sync.dma_start(wk_sb[:, kc], wk_r[:, kc])
            nc.sync.dma_start(xk_sb[:, kc], xk_r[:, kc])
        # V / wo loads follow K's on the same queue: issue order = arrival
        # order, so the K projection is never bandwidth-starved at startup
        wv_sb = wxp.tile([P, NT, H * DV], BF, tag="w", name="wv_sb")
        xv_sb = wxp.tile([P, NT, S], BF, tag="x", name="xv_sb")
        for kc in range(NT):
            nc.sync.dma_start(wv_sb[:, kc], wv_r[:, kc])
            nc.sync.dma_start(xv_sb[:, kc], xv_r[:, kc])
        nc.sync.dma_start(wo_sb, wo_r)

        # K projection runs 8 chains wide across every PSUM bank, kc-major:
        # during the initial DMA ramp each arriving wk/xk tile feeds 8
        # matmuls, so the PE streams from the first tile instead of idling
        # behind a 2-chain window.
        for r in range(2):
            pss = []
            for c in range(4):
                pr = ps_pair.tile([P, 2 * CH], FP, tag="pair", name=f"kp_{r}_{c}")
                pss.append(pr[:, 0:CH])
                pss.append(pr[:, CH : 2 * CH])
            # chains c=0..7 -> (m, n) = (r*4 + c//2, c%2)
            for kc in range(NT):
                for c in range(8):
                    m, n = r * 4 + c // 2, c % 2
                    nc.tensor.matmul(
                        pss[c],
                        wk_sb[:, kc, m * P : (m + 1) * P],
                        xk_sb[:, kc, n * CH : (n + 1) * CH],
                        start=(kc == 0),
                        stop=(kc == NT - 1),
                    )
            for c in range(8):
                m, n = r * 4 + c // 2, c % 2
                dst = kT_sb[:, m, n * CH : (n + 1) * CH]
                if zero_bias:
                    evac_copy(dst, pss[c])
                else:
                    nc.vector.tensor_scalar_add(dst, pss[c], bk_sb[:, m : m + 1])

        wq_sb = wxp.tile([P, NT, H * DK], BF, tag="w", name="wq_sb")
        xq_sb = wxp.tile([P, NT, S], BF, tag="x", name="xq_sb")
        for kc in range(NT):
            nc.sync.dma_start(wq_sb[:, kc], wq_r[:, kc])
            nc.sync.dma_start(xq_sb[:, kc], xq_r[:, kc])

        # ---------------- V projection (out[s, hv] = xv.T @ wv) -----------
        for m in range(NT):
            for n in range(NCH):
                ps = ps_pv.tile([P, CH], FP, tag="c512", name=f"vproj_{m}_{n}")
                for kc in range(NT):
                    nc.tensor.matmul(
                        ps,
                        xv_sb[:, kc, m * P : (m + 1) * P],
                        wv_sb[:, kc, n * CH : (n + 1) * CH],
                        start=(kc == 0),
                        stop=(kc == NT - 1),
                    )
                dst = v_sb[:, m, 8 * n : 8 * (n + 1), DV:P]
                src = ps.rearrange("p (h v) -> p h v", v=DV)
                if zero_bias:
                    nc.vector.tensor_copy(out=dst, in_=src)
                else:
                    nc.vector.tensor_tensor(
                        dst,
                        src,
                        bv_sb[:, n * CH : (n + 1) * CH].rearrange(
                            "p (h v) -> p h v", v=DV
                        ),
                        ALU.add,
                    )

        # ---------------- Q projection, q-chunk n ----------------
        def qproj_mtile(m, n, pool):
            ps = proj_chain(wq_sb, xq_sb, m, n, pool, f"qproj_{m}_{n}")
            dst = qT_sb[:, m, n * CH : (n + 1) * CH]
            if zero_bias:
                evac_copy(dst, ps, act_ok=(n == 0))
            else:
                nc.vector.tensor_scalar_add(dst, ps, bq_sb[:, m : m + 1])

        for m in range(NT):
            qproj_mtile(m, 0, ps_pv)

        # Filler steps: independent dense-matmul half-chains threaded into
        # the attention instruction stream at its natural stall points (the
        # PE queue is in-order, so filler AFTER a stalled PV never runs).
        class Feeder:
            def __init__(self):
                self.steps = []

            def pull(self, k=1):
                for _ in range(k):
                    if self.steps:
                        self.steps.pop(0)()

            def drain(self):
                while self.steps:
                    self.steps.pop(0)()

        # ---------------- attention head (q-chunk jc) ----------------
        def attn_head(h, jc, feeder=None):
            hp, sub = h // 2, h % 2
            qb = sub * DK  # partition base of this head's q/k rows
            groups = _st_groups(jc)
            pchs = []
            for g, group in enumerate(groups):
                gw = group[-1][1] + group[-1][2]
                pack = ps_pair.tile(
                    [P, 2 * CH], FP, tag="pair", name=f"st_{h}_{jc}_{g}"
                )
                for i, seg, w in group:
                    off = max(0, i * P - jc * CH)
                    nc.tensor.matmul(
                        pack[:, seg : seg + w],
                        kT_sb[qb : qb + DK, hp, i * P : (i + 1) * P],
                        qT_sb[qb : qb + DK, hp, jc * CH + off : (jc + 1) * CH],
                        start=True,
                        stop=True,
                    )
                pch = pchp.tile([P, 2 * CH], BF, tag="p", name=f"p_{h}_{jc}_{g}")
                nc.scalar.activation(pch[:, :gw], pack[:, :gw], AFT.Exp)
                for i, seg, w in group:
                    if i * P >= jc * CH:  # tile contains the diagonal
                        nc.vector.tensor_tensor(
                            pch[:, seg : seg + P],
                            pch[:, seg : seg + P],
                            tri_sb,
                            ALU.mult,
                        )
                pchs.append(pch)
            if feeder is not None:
                feeder.pull()
            pos = ps_pv.tile([P, CH], FP, tag="c512", name=f"po_{h}_{jc}")
            flat = [(i, seg, w, g) for g, grp in enumerate(groups) for i, seg, w in grp]
            flat.sort()
            for idx, (i, seg, w, g) in enumerate(flat):
                off = max(0, i * P - jc * CH)
                nc.tensor.matmul(
                    pos[:, off:],
                    v_sb[:, i, h, :],
                    pchs[g][:, seg : seg + w],
                    start=(idx == 0),
                    stop=(idx == len(flat) - 1),
                )
            if feeder is not None:
                feeder.pull()
            # normalization: denominator sits on PSUM partition 0 (custom-DVE
            # reciprocal and gpsimd broadcast only work from partition 0),
            # O^T rows on partitions 64:128.
            rinv = smalls.tile([1, CH], FP, tag="rinv")
            nc.vector.reciprocal_approx_fast(out=rinv, in_=pos[0:1, :])
            rrep = smalls.tile([DV, CH], FP, tag="rrep")
            nc.gpsimd.partition_broadcast(rrep, rinv)
            nc.vector.tensor_tensor(
                oT_sb[qb : qb + DV, hp, jc * CH : (jc + 1) * CH],
                pos[DV:P],
                rrep,
                ALU.mult,
            )

        # ---------------- output projection m-tile ----------------
        def outproj_mtile(m):
            for n in range(NCH):
                py = ps_out.tile([P, CH], FP, tag="c512", name=f"py_{m}_{n}")
                for kc in range(NT):
                    nc.tensor.matmul(
                        py,
                        oT_sb[:, kc, m * P : (m + 1) * P],
                        wo_sb[:, kc, n * CH : (n + 1) * CH],
                        start=(kc == 0),
                        stop=(kc == NT - 1),
                    )
                yt = ypool.tile([P, CH], FP, tag="y")
                if zero_bias:
                    evac_copy(yt, py)
                else:
                    nc.vector.tensor_tensor(
                        yt, py, borep_sb[:, n * CH : (n + 1) * CH], ALU.add
                    )
                nc.sync.dma_start(y_r[:, m, n * CH : (n + 1) * CH], yt)

        def chain_halves(feeder, lhs_sb, rhs_sb, m, n, dst_fn, name):
            st = {}

            def s1():
                ps = ps_out.tile([P, CH], FP, tag="c512", name=name)
                st["ps"] = ps
                for kc in range(4):
                    nc.tensor.matmul(
                        ps,
                        lhs_sb[:, kc, m * P : (m + 1) * P],
                        rhs_sb[:, kc, n * CH : (n + 1) * CH],
                        start=(kc == 0),
                        stop=False,
                    )

            def s2():
                ps = st["ps"]
                for kc in range(4, NT):
                    nc.tensor.matmul(
                        ps,
                        lhs_sb[:, kc, m * P : (m + 1) * P],
                        rhs_sb[:, kc, n * CH : (n + 1) * CH],
                        start=False,
                        stop=(kc == NT - 1),
                    )
                dst_fn(ps)

            feeder.steps.append(s1)
            feeder.steps.append(s2)

        def qproj_evac(m):
            def fn(ps):
                dst = qT_sb[:, m, CH : 2 * CH]
                if zero_bias:
                    evac_copy(dst, ps)
                else:
                    nc.vector.tensor_scalar_add(dst, ps, bq_sb[:, m : m + 1])

            return fn

        def outproj_evac(m, n):
            def fn(ps):
                yt = ypool.tile([P, CH], FP, tag="y")
                if zero_bias:
                    evac_copy(yt, ps)
                else:
                    nc.vector.tensor_tensor(
                        yt, ps, borep_sb[:, n * CH : (n + 1) * CH], ALU.add
                    )
                nc.sync.dma_start(y_r[:, m, n * CH : (n + 1) * CH], yt)

            return fn

        # q-chunk 0 attention, Q-proj chunk 1 threaded into the PE stream
        feed0 = Feeder()
        for m in range(NT):
            chain_halves(feed0, wq_sb, xq_sb, m, 1, qproj_evac(m), f"qp1_{m}")
        for h in range(H):
            attn_head(h, 0, feed0)
        feed0.drain()
        # q-chunk 1 attention, output-proj chunk 0 threaded in
        feed1 = Feeder()
        for m in range(4):
            for n in range(NCH):
                chain_halves(
                    feed1, oT_sb, wo_sb, m, n, outproj_evac(m, n), f"op_{m}_{n}"
                )
        for h in range(H):
            attn_head(h, 1, feed1)
        feed1.drain()
        for m in range(4, NT):
            outproj_mtile(m)

    nc.finalize()
    return nc


_NC_CACHE = {}


def _get_nc(zero_bias: bool) -> bass.Bass:
    if zero_bias not in _NC_CACHE:
        _NC_CACHE[zero_bias] = build_nc(zero_bias)
    return _NC_CACHE[zero_bias]


def prep_shared(Wq, bq, Wk, bk, Wv, bv, Wo, bo, zero_bias):
    """Host-side packing of weights/biases (shared by all cores)."""
    scale = 1.0 / math.sqrt(DK)
    Wq = np.asarray(Wq, np.float32)
    Wk = np.asarray(Wk, np.float32)
    Wv = np.asarray(Wv, np.float32)
    Wo = np.asarray(Wo, np.float32)
    out = {
        "wq": np.ascontiguousarray(
            (Wq.transpose(1, 0, 2).reshape(C, H * DK) * scale).astype(BF_NP)
        ),
        "wk": np.ascontiguousarray(
            Wk.transpose(1, 0, 2).reshape(C, H * DK).astype(BF_NP)
        ),
        "wv": np.ascontiguousarray(
            Wv.transpose(1, 0, 2).reshape(C, H * DV).astype(BF_NP)
        ),
        "wo": Wo.astype(BF_NP),
    }
    if not zero_bias:
        out.update(
            {
                "bq": np.ascontiguousarray(
                    (np.asarray(bq, np.float32).reshape(H * DK) * scale)
                    .reshape(NT, P)
                    .T.astype(np.float32)
                ),
                "bk": np.ascontiguousarray(
                    np.asarray(bk, np.float32).reshape(NT, P).T.astype(np.float32)
                ),
                "bv": np.ascontiguousarray(
                    np.broadcast_to(
                        np.asarray(bv, np.float32).reshape(1, H * DV), (P, H * DV)
                    ).astype(np.float32)
                ),
                "bo": np.ascontiguousarray(np.asarray(bo, np.float32).reshape(1, C)),
            }
        )
    return out


def prep_core(q_embs_b, k_embs_b, v_embs_b):
    return {
        "xq": np.ascontiguousarray(np.asarray(q_embs_b, np.float32).T.astype(BF_NP)),
        "xk": np.ascontiguousarray(np.asarray(k_embs_b, np.float32).T.astype(BF_NP)),
        "xv": np.ascontiguousarray(np.asarray(v_embs_b, np.float32).T.astype(BF_NP)),
    }


def kernel(q_embs, k_embs, v_embs, Wq, bq, Wk, bk, Wv, bv, Wo, bo, **run_kwargs):
    zero_bias = all(
        not np.any(np.asarray(b)) for b in (bq, bk, bv, bo)
    )
    nc = _get_nc(zero_bias)
    shared = prep_shared(Wq, bq, Wk, bk, Wv, bv, Wo, bo, zero_bias)
    q_embs = np.asarray(q_embs, np.float32)
    k_embs = np.asarray(k_embs, np.float32)
    v_embs = np.asarray(v_embs, np.float32)
    in_maps = []
    for b in range(B):
        m = dict(shared)
        m.update(prep_core(q_embs[b], k_embs[b], v_embs[b]))
        in_maps.append(m)
    res = run_bass_kernel_spmd(nc, in_maps, core_ids=list(range(B)), **run_kwargs)
    out = np.stack([res.results[i]["y"] for i in range(B)], axis=0)
    if run_kwargs:
        kernel.last_results = res
    return out


if __name__ == "__main__":
    rng = np.random.default_rng(0)
    inputs = {
        "q_embs": rng.standard_normal((B, S, C), np.float32),
        "k_embs": rng.standard_normal((B, S, C), np.float32),
        "v_embs": rng.standard_normal((B, S, C), np.float32),
        "Wq": rng.standard_normal((H, C, DK), np.float32) * 0.02,
        "bq": np.zeros((H, DK), np.float32),
        "Wk": rng.standard_normal((H, C, DK), np.float32) * 0.02,
        "bk": np.zeros((H, DK), np.float32),
        "Wv": rng.standard_normal((H, C, DV), np.float32) * 0.02,
        "bv": np.zeros((H, DV), np.float32),
        "Wo": rng.standard_normal((H * DV, C), np.float32) * 0.02,
        "bo": np.zeros((C,), np.float32),
    }
    out = kernel(**inputs)
    print(out.shape, out.dtype)
